# revision 15
# baseline (speedup 1.0000x reference)
"""Chamfer distance loss kernel for Trainium2 (8 NeuronCores, SPMD).

Problem: nn_ChamferDistLoss — inputs pt_src, pt_ref, points_src, points_ref,
all [B=4, N=4096, 3] fp32.  Output: scalar

    loss = chamfer(pt_src, pt_ref)            # symmetric
         + chamfer_single(pt_src, points_src) # one-sided
         + chamfer_single(pt_ref, points_ref) # one-sided

Every term decomposes into one-sided tasks "sum_i min_j ||a_i - b_j||" over
[4096 x 4096] pairs.  There are 16 (direction, batch) tasks; each core gets 2.

Device algorithm per task (A, B both [4096, 3]):
  d2[i,j] = a2_i + b2_j - 2<a_i,b_j> is computed by ONE K=16 fp16 matmul per
  [128 x 512] tile, using an exact hi/lo fp16 split of the fp32 data (all 4
  cross terms kept), accumulated in fp32 PSUM:
    A rows: [xh yh zh xl yl zl | xh yh zh xl yl zl | 1 1 | a2h a2l]
    B rows: [-2xh..-2zh -2xl..-2zl | -2xl..-2zl -2xh..-2zh | b2h b2l | 1 1]
  Then row-min over chunks (DVE reduce min), relu, sqrt, sum, and a
  partition-sum via ones-matmul -> per-task scalar.  Host: sum 16 / 16384.
"""

import numpy as np

import concourse.bass as bass
import concourse.bacc as bacc
import concourse.tile as tile
from concourse import mybir
from concourse import bass_utils

F32 = mybir.dt.float32
F16 = mybir.dt.float16
AX = mybir.AxisListType
OP = mybir.AluOpType
ACT = mybir.ActivationFunctionType

NPTS = 4096
P = 128
GRP = NPTS // P          # 32 points per partition in the [128, 96] load
NBLK = NPTS // P         # 32 M-blocks per task
CHUNK = 512              # matmul moving free dim
PSUM_FD = 1024           # psum tile free dim (2 banks)
NTILE = NPTS // PSUM_FD  # 4 psum tiles per m-block
NTASK = 2                # tasks per core
K = 16                   # matmul contraction rows


def _row_dma(nc, R, r, src):
    """DMA one [128, 32] SBUF block into row r of R ([1, 4096], n = p*32+c)."""
    nc.gpsimd.dma_start(
        R[r : r + 1, :].rearrange("r (p c) -> r p c", p=P), src
    )


def _build_side(tc, pool, wpool, t_dram, ones_ap, side, tag):
    """Load one [4096, 3] input, build its [16, 4096] fp16 matmul operand."""
    nc = tc.nc
    L = wpool.tile([P, 3 * GRP], F32, tag="ld")
    # contiguous per-partition load: partition p holds points p*32 .. p*32+31
    nc.sync.dma_start(L[:], t_dram.rearrange("(p g) k -> p (g k)", p=P))

    # coordinate source in (k, g) order; b side scaled by -2 (exact in fp16)
    if side == "a":
        base_kg = L[:].rearrange("p (g k) -> p k g", k=3)
    else:
        t2 = wpool.tile([P, 3 * GRP], F32, tag="t2")
        nc.vector.tensor_scalar_mul(t2[:], L[:], -2.0)
        base_kg = t2[:].rearrange("p (g k) -> p k g", k=3)

    # hi/lo fp16 split of the (scaled) coordinates, blocks [xh yh zh xl yl zl]
    S6 = wpool.tile([P, 6 * GRP], F16, tag="s6")
    hi3 = S6[:, 0 : 3 * GRP].rearrange("p (k g) -> p k g", k=3)
    lo3 = S6[:, 3 * GRP : 6 * GRP].rearrange("p (k g) -> p k g", k=3)
    nc.vector.tensor_copy(hi3, base_kg)
    nc.vector.tensor_sub(lo3, base_kg, hi3)

    # squared norm n2 = x^2+y^2+z^2 (fp32), then hi/lo split
    sq = wpool.tile([P, 3 * GRP], F32, tag="sq")
    nc.vector.tensor_mul(sq[:], L[:], L[:])
    sq_kg = sq[:].rearrange("p (g k) -> p k g", k=3)
    n2 = wpool.tile([P, GRP], F32, tag="n2")
    nc.vector.tensor_add(n2[:], sq_kg[:, 0:1, :], sq_kg[:, 1:2, :])
    nc.vector.tensor_add(n2[:], n2[:], sq_kg[:, 2:3, :])
    Sn = wpool.tile([P, 2 * GRP], F16, tag="sn")
    nc.vector.tensor_copy(Sn[:, 0:GRP], n2[:])
    nc.vector.tensor_sub(Sn[:, GRP : 2 * GRP], n2[:], Sn[:, 0:GRP])

    # assemble R [16, 4096] fp16; free index n = p*32 + c
    R = pool.tile([K, NPTS], F16, tag=tag)
    blk = lambda i: S6[:, i * GRP : (i + 1) * GRP]
    if side == "a":
        for r in range(6):
            _row_dma(nc, R, r, blk(r))            # rows 0-5:  h h h l l l
        for r in range(6):
            _row_dma(nc, R, 6 + r, blk(r))        # rows 6-11: h h h l l l
        nc.sync.dma_start(R[12:14, :], ones_ap)   # rows 12-13: 1 1
        _row_dma(nc, R, 14, Sn[:, 0:GRP])         # row 14: a2h
        _row_dma(nc, R, 15, Sn[:, GRP : 2 * GRP])  # row 15: a2l
    else:
        for r in range(6):
            _row_dma(nc, R, r, blk(r))            # rows 0-5:  h h h l l l
        for r in range(3):
            _row_dma(nc, R, 6 + r, blk(3 + r))    # rows 6-8:  l l l
        for r in range(3):
            _row_dma(nc, R, 9 + r, blk(r))        # rows 9-11: h h h
        _row_dma(nc, R, 12, Sn[:, 0:GRP])         # row 12: b2h
        _row_dma(nc, R, 13, Sn[:, GRP : 2 * GRP])  # row 13: b2l
        nc.sync.dma_start(R[14:16, :], ones_ap)   # rows 14-15: 1 1
    return R


def chamfer_core_kernel(tc, out_ap, in_aps, ones_ap):
    """Per-core program: 2 tasks, each sum_i min_j ||a_i - b_j||."""
    nc = tc.nc
    from contextlib import ExitStack

    with ExitStack() as ctx:
        const_pool = ctx.enter_context(tc.tile_pool(name="const", bufs=1))
        work_pool = ctx.enter_context(tc.tile_pool(name="work", bufs=4))
        psum_pool = ctx.enter_context(tc.tile_pool(name="psum", bufs=3, space="PSUM"))
        psum_fin_pool = ctx.enter_context(
            tc.tile_pool(name="psum_fin", bufs=1, space="PSUM")
        )
        small_pool = ctx.enter_context(tc.tile_pool(name="small", bufs=4))

        sides = []
        for t in range(NTASK):
            Ra = _build_side(
                tc, const_pool, work_pool, in_aps[2 * t], ones_ap, "a", f"Ra{t}"
            )
            Rb = _build_side(
                tc, const_pool, work_pool, in_aps[2 * t + 1], ones_ap, "b", f"Rb{t}"
            )
            sides.append((Ra, Rb))

        # collect[p, t*32 + m] = min_j d2 for a-point (m*128 + p) of task t
        collect = const_pool.tile([P, NTASK * NBLK], F32)

        for t in range(NTASK):
            Ra, Rb = sides[t]
            for m in range(NBLK):
                lhsT = Ra[:, m * P : (m + 1) * P]
                minv = small_pool.tile([P, NTILE], F32, tag="minv")
                for i in range(NTILE):
                    ps = psum_pool.tile([P, PSUM_FD], F32, tag="ps")
                    for h in range(PSUM_FD // CHUNK):
                        n0 = i * PSUM_FD + h * CHUNK
                        nc.tensor.matmul(
                            ps[:, h * CHUNK : (h + 1) * CHUNK],
                            lhsT,
                            Rb[:, n0 : n0 + CHUNK],
                            start=True,
                            stop=True,
                        )
                    nc.vector.tensor_reduce(
                        minv[:, i : i + 1], ps[:], axis=AX.X, op=OP.min
                    )
                nc.vector.tensor_reduce(
                    collect[:, t * NBLK + m : t * NBLK + m + 1],
                    minv[:],
                    axis=AX.X,
                    op=OP.min,
                )

        # d = sqrt(relu(d2)); per-task per-partition sums
        dists = const_pool.tile([P, NTASK * NBLK], F32)
        nc.vector.tensor_scalar_max(dists[:], collect[:], 0.0)
        nc.scalar.activation(dists[:], dists[:], ACT.Sqrt)

        psums = small_pool.tile([P, NTASK], F32, tag="psums")
        for t in range(NTASK):
            nc.vector.tensor_reduce(
                psums[:, t : t + 1],
                dists[:, t * NBLK : (t + 1) * NBLK],
                axis=AX.X,
                op=OP.add,
            )

        # partition sum via ones-matmul: [1, NTASK] = ones[128,1].T @ psums
        ones = small_pool.tile([P, 1], F32, tag="ones")
        nc.vector.memset(ones[:], 1.0)
        ps_out = psum_fin_pool.tile([1, CHUNK], F32, tag="ps_fin")
        nc.tensor.matmul(
            ps_out[:, 0:NTASK], ones[:], psums[:], start=True, stop=True
        )
        outs = small_pool.tile([1, NTASK], F32, tag="outs")
        nc.vector.tensor_copy(outs[:], ps_out[:, 0:NTASK])
        nc.gpsimd.dma_start(out_ap, outs[:])


_CACHED = {}


def _get_program(repeats=1):
    if repeats in _CACHED:
        return _CACHED[repeats]
    nc = bacc.Bacc("TRN2", target_bir_lowering=False, debug=False, num_devices=8)
    in_names = ["a0", "b0", "a1", "b1"]
    in_aps = [
        nc.dram_tensor(n, [NPTS, 3], F32, kind="ExternalInput").ap() for n in in_names
    ]
    ones_ap = nc.dram_tensor("ones2", [2, NPTS], F16, kind="ExternalInput").ap()
    out_ap = nc.dram_tensor("out", [1, NTASK], F32, kind="ExternalOutput").ap()
    with tile.TileContext(nc) as tc:
        for _ in range(repeats):
            chamfer_core_kernel(tc, out_ap, in_aps, ones_ap)
    nc.compile()
    _CACHED[repeats] = nc
    return nc


def _shard(pt_src, pt_ref, points_src, points_ref):
    """Host-side sharding: 16 (direction, batch) tasks -> 8 cores x 2 tasks."""
    ones2 = np.ones((2, NPTS), dtype=np.float16)
    in_maps = []
    for c in range(8):
        if c < 4:
            b = c
            m = {"a0": pt_src[b], "b0": pt_ref[b], "a1": pt_ref[b], "b1": pt_src[b]}
        else:
            b = c - 4
            m = {
                "a0": pt_src[b],
                "b0": points_src[b],
                "a1": pt_ref[b],
                "b1": points_ref[b],
            }
        m = {k: np.ascontiguousarray(v, dtype=np.float32) for k, v in m.items()}
        m["ones2"] = ones2
        in_maps.append(m)
    return in_maps


def kernel(pt_src, pt_ref, points_src, points_ref, _results_hook=None, _repeats=1):
    nc = _get_program(_repeats)
    in_maps = _shard(pt_src, pt_ref, points_src, points_ref)
    res = bass_utils.run_bass_kernel_spmd(nc, in_maps, core_ids=list(range(8)))
    if _results_hook is not None:
        _results_hook(res)
    total = np.float64(0.0)
    for r in res.results:
        total += np.float64(r["out"].astype(np.float64).sum())
    out = np.float32(total / (4 * 4096))
    return np.asarray(out, dtype=np.float32)


# revision 17
# speedup vs baseline: 182.6957x; 182.6957x over previous
"""Chamfer distance loss kernel for Trainium2 (8 NeuronCores, SPMD).

Problem: nn_ChamferDistLoss — inputs pt_src, pt_ref, points_src, points_ref,
all [B=4, N=4096, 3] fp32.  Output: scalar

    loss = chamfer(pt_src, pt_ref)            # symmetric
         + chamfer_single(pt_src, points_src) # one-sided
         + chamfer_single(pt_ref, points_ref) # one-sided

Every term decomposes into one-sided tasks "sum_i min_j ||a_i - b_j||" over
[4096 x 4096] pairs.  There are 16 (direction, batch) tasks; each core gets 2.

Device algorithm per task (A, B both [4096, 3]):
  d2[i,j] = a2_i + b2_j - 2<a_i,b_j> is computed by ONE K=16 fp16 matmul per
  [128 x 512] tile, using an exact hi/lo fp16 split of the fp32 data (all 4
  cross terms kept), accumulated in fp32 PSUM:
    A rows: [xh yh zh xl yl zl | xh yh zh xl yl zl | 1 1 | a2h a2l]
    B rows: [-2xh..-2zh -2xl..-2zl | -2xl..-2zl -2xh..-2zh | b2h b2l | 1 1]
  Then row-min over chunks (DVE reduce min), relu, sqrt, sum, and a
  partition-sum via ones-matmul -> per-task scalar.  Host: sum 16 / 16384.
"""

import numpy as np

import concourse.bass as bass
import concourse.bacc as bacc
import concourse.tile as tile
from concourse import mybir
from concourse import bass_utils

F32 = mybir.dt.float32
F16 = mybir.dt.float16
AX = mybir.AxisListType
OP = mybir.AluOpType
ACT = mybir.ActivationFunctionType

NPTS = 4096
P = 128
GRP = NPTS // P          # 32 points per partition in the [128, 96] load
NBLK = NPTS // P         # 32 M-blocks per task
CHUNK = 512              # matmul moving free dim
PSUM_FD = 1024           # psum tile free dim (2 banks)
NTILE = NPTS // PSUM_FD  # 4 psum tiles per m-block
NTASK = 2                # tasks per core
K = 16                   # matmul contraction rows


def _row_dma(nc, R, r, src):
    """DMA one [128, 32] SBUF block into row r of R ([1, 4096], n = p*32+c)."""
    nc.gpsimd.dma_start(
        R[r : r + 1, :].rearrange("r (p c) -> r p c", p=P), src
    )


def _build_side(tc, pool, wpool, t_dram, ones_ap, side, tag):
    """Load one [4096, 3] input, build its [16, 4096] fp16 matmul operand."""
    nc = tc.nc
    L = wpool.tile([P, 3 * GRP], F32, tag="ld")
    # contiguous per-partition load: partition p holds points p*32 .. p*32+31
    nc.sync.dma_start(L[:], t_dram.rearrange("(p g) k -> p (g k)", p=P))

    # coordinate source in (k, g) order; b side scaled by -2 (exact in fp16)
    if side == "a":
        base_kg = L[:].rearrange("p (g k) -> p k g", k=3)
    else:
        t2 = wpool.tile([P, 3 * GRP], F32, tag="t2")
        nc.vector.tensor_scalar_mul(t2[:], L[:], -2.0)
        base_kg = t2[:].rearrange("p (g k) -> p k g", k=3)

    # hi/lo fp16 split of the (scaled) coordinates, blocks [xh yh zh xl yl zl]
    S6 = wpool.tile([P, 6 * GRP], F16, tag="s6")
    hi3 = S6[:, 0 : 3 * GRP].rearrange("p (k g) -> p k g", k=3)
    lo3 = S6[:, 3 * GRP : 6 * GRP].rearrange("p (k g) -> p k g", k=3)
    nc.vector.tensor_copy(hi3, base_kg)
    nc.vector.tensor_sub(lo3, base_kg, hi3)

    # squared norm n2 = x^2+y^2+z^2 (fp32), then hi/lo split
    sq = wpool.tile([P, 3 * GRP], F32, tag="sq")
    nc.vector.tensor_mul(sq[:], L[:], L[:])
    sq_kg = sq[:].rearrange("p (g k) -> p k g", k=3)
    n2 = wpool.tile([P, GRP], F32, tag="n2")
    nc.vector.tensor_add(n2[:], sq_kg[:, 0:1, :], sq_kg[:, 1:2, :])
    nc.vector.tensor_add(n2[:], n2[:], sq_kg[:, 2:3, :])
    Sn = wpool.tile([P, 2 * GRP], F16, tag="sn")
    nc.vector.tensor_copy(Sn[:, 0:GRP], n2[:])
    nc.vector.tensor_sub(Sn[:, GRP : 2 * GRP], n2[:], Sn[:, 0:GRP])

    # assemble R [16, 4096] fp16; free index n = p*32 + c
    R = pool.tile([K, NPTS], F16, tag=tag)
    blk = lambda i: S6[:, i * GRP : (i + 1) * GRP]
    if side == "a":
        for r in range(6):
            _row_dma(nc, R, r, blk(r))            # rows 0-5:  h h h l l l
        for r in range(6):
            _row_dma(nc, R, 6 + r, blk(r))        # rows 6-11: h h h l l l
        nc.sync.dma_start(R[12:14, :], ones_ap)   # rows 12-13: 1 1
        _row_dma(nc, R, 14, Sn[:, 0:GRP])         # row 14: a2h
        _row_dma(nc, R, 15, Sn[:, GRP : 2 * GRP])  # row 15: a2l
    else:
        for r in range(6):
            _row_dma(nc, R, r, blk(r))            # rows 0-5:  h h h l l l
        for r in range(3):
            _row_dma(nc, R, 6 + r, blk(3 + r))    # rows 6-8:  l l l
        for r in range(3):
            _row_dma(nc, R, 9 + r, blk(r))        # rows 9-11: h h h
        _row_dma(nc, R, 12, Sn[:, 0:GRP])         # row 12: b2h
        _row_dma(nc, R, 13, Sn[:, GRP : 2 * GRP])  # row 13: b2l
        nc.sync.dma_start(R[14:16, :], ones_ap)   # rows 14-15: 1 1
    return R


def chamfer_core_kernel(tc, out_ap, in_aps, ones_ap):
    """Per-core program: 2 tasks, each sum_i min_j ||a_i - b_j||."""
    nc = tc.nc
    from contextlib import ExitStack

    with ExitStack() as ctx:
        const_pool = ctx.enter_context(tc.tile_pool(name="const", bufs=1))
        work_pool = ctx.enter_context(tc.tile_pool(name="work", bufs=4))
        psum_pool = ctx.enter_context(tc.tile_pool(name="psum", bufs=3, space="PSUM"))
        psum_fin_pool = ctx.enter_context(
            tc.tile_pool(name="psum_fin", bufs=1, space="PSUM")
        )
        small_pool = ctx.enter_context(tc.tile_pool(name="small", bufs=4))

        sides = []
        for t in range(NTASK):
            Ra = _build_side(
                tc, const_pool, work_pool, in_aps[2 * t], ones_ap, "a", f"Ra{t}"
            )
            Rb = _build_side(
                tc, const_pool, work_pool, in_aps[2 * t + 1], ones_ap, "b", f"Rb{t}"
            )
            sides.append((Ra, Rb))

        # collect[p, t*32 + m] = min_j d2 for a-point (m*128 + p) of task t
        collect = const_pool.tile([P, NTASK * NBLK], F32)

        for t in range(NTASK):
            Ra, Rb = sides[t]
            for m in range(NBLK):
                lhsT = Ra[:, m * P : (m + 1) * P]
                minv = small_pool.tile([P, NTILE], F32, tag="minv")
                for i in range(NTILE):
                    ps = psum_pool.tile([P, PSUM_FD], F32, tag="ps")
                    for h in range(PSUM_FD // CHUNK):
                        n0 = i * PSUM_FD + h * CHUNK
                        nc.tensor.matmul(
                            ps[:, h * CHUNK : (h + 1) * CHUNK],
                            lhsT,
                            Rb[:, n0 : n0 + CHUNK],
                            start=True,
                            stop=True,
                        )
                    nc.vector.tensor_reduce(
                        minv[:, i : i + 1], ps[:], axis=AX.X, op=OP.min
                    )
                nc.vector.tensor_reduce(
                    collect[:, t * NBLK + m : t * NBLK + m + 1],
                    minv[:],
                    axis=AX.X,
                    op=OP.min,
                )

        # d = sqrt(relu(d2)); per-task per-partition sums
        dists = const_pool.tile([P, NTASK * NBLK], F32)
        nc.vector.tensor_scalar_max(dists[:], collect[:], 0.0)
        nc.scalar.activation(dists[:], dists[:], ACT.Sqrt)

        psums = small_pool.tile([P, NTASK], F32, tag="psums")
        for t in range(NTASK):
            nc.vector.tensor_reduce(
                psums[:, t : t + 1],
                dists[:, t * NBLK : (t + 1) * NBLK],
                axis=AX.X,
                op=OP.add,
            )

        # partition sum via ones-matmul: [1, NTASK] = ones[128,1].T @ psums
        ones = small_pool.tile([P, 1], F32, tag="ones")
        nc.vector.memset(ones[:], 1.0)
        ps_out = psum_fin_pool.tile([1, CHUNK], F32, tag="ps_fin")
        nc.tensor.matmul(
            ps_out[:, 0:NTASK], ones[:], psums[:], start=True, stop=True
        )
        outs = small_pool.tile([1, NTASK], F32, tag="outs")
        nc.vector.tensor_copy(outs[:], ps_out[:, 0:NTASK])
        nc.gpsimd.dma_start(out_ap, outs[:])


_CACHED = {}


def _get_program(repeats=1):
    if repeats in _CACHED:
        return _CACHED[repeats]
    nc = bacc.Bacc("TRN2", target_bir_lowering=False, debug=False, num_devices=8)
    in_names = ["a0", "b0", "a1", "b1"]
    in_aps = [
        nc.dram_tensor(n, [NPTS, 3], F32, kind="ExternalInput").ap() for n in in_names
    ]
    ones_ap = nc.dram_tensor("ones2", [2, NPTS], F16, kind="ExternalInput").ap()
    out_ap = nc.dram_tensor("out", [1, NTASK], F32, kind="ExternalOutput").ap()
    with tile.TileContext(nc) as tc:
        for _ in range(repeats):
            chamfer_core_kernel(tc, out_ap, in_aps, ones_ap)
    nc.compile()
    _CACHED[repeats] = nc
    return nc


def _shard(pt_src, pt_ref, points_src, points_ref):
    """Host-side sharding: 16 (direction, batch) tasks -> 8 cores x 2 tasks."""
    ones2 = np.ones((2, NPTS), dtype=np.float16)
    in_maps = []
    for c in range(8):
        if c < 4:
            b = c
            m = {"a0": pt_src[b], "b0": pt_ref[b], "a1": pt_ref[b], "b1": pt_src[b]}
        else:
            b = c - 4
            m = {
                "a0": pt_src[b],
                "b0": points_src[b],
                "a1": pt_ref[b],
                "b1": points_ref[b],
            }
        m = {k: np.ascontiguousarray(v, dtype=np.float32) for k, v in m.items()}
        m["ones2"] = ones2
        in_maps.append(m)
    return in_maps


def _get_runner(repeats=1):
    """Cached jitted executor — the NEFF is loaded once; later calls only
    dispatch an execute (unlike run_bass_kernel_spmd, which rebuilds the
    jit closure and re-loads the NEFF on every call)."""
    key = ("runner", repeats)
    if key in _CACHED:
        return _CACHED[key]
    import jax
    from jax.sharding import Mesh, PartitionSpec
    from jax.experimental.shard_map import shard_map
    from concourse import bass2jax, mybir as _mb

    bass2jax.install_neuronx_cc_hook()
    nc = _get_program(repeats)
    n_cores = 8

    partition_name = (
        nc.partition_id_tensor.name if nc.partition_id_tensor is not None else None
    )
    in_names, out_names, out_avals, zero_shapes = [], [], [], []
    for alloc in nc.m.functions[0].allocations:
        if not isinstance(alloc, _mb.MemoryLocationSet):
            continue
        name = alloc.memorylocations[0].name
        if alloc.kind == "ExternalInput":
            if name != partition_name:
                in_names.append(name)
        elif alloc.kind == "ExternalOutput":
            out_names.append(name)
            shape = tuple(alloc.tensor_shape)
            dtype = _mb.dt.np(alloc.dtype)
            out_avals.append(jax.core.ShapedArray(shape, dtype))
            zero_shapes.append((shape, dtype))
    n_params = len(in_names)
    all_names = in_names + out_names
    if partition_name is not None:
        all_names = all_names + [partition_name]
    donate = tuple(range(n_params, n_params + len(out_names)))

    def _body(*args):
        operands = list(args)
        if partition_name is not None:
            operands.append(bass2jax.partition_id_tensor())
        outs = bass2jax._bass_exec_p.bind(
            *operands,
            out_avals=tuple(out_avals),
            in_names=tuple(all_names),
            out_names=tuple(out_names),
            lowering_input_output_aliases=(),
            sim_require_finite=True,
            sim_require_nnan=True,
            nc=nc,
        )
        return tuple(outs)

    devices = jax.devices()[:n_cores]
    mesh = Mesh(np.asarray(devices), ("core",))
    in_specs = (PartitionSpec("core"),) * (n_params + len(out_names))
    out_specs = (PartitionSpec("core"),) * len(out_names)
    sharded = jax.jit(
        shard_map(
            _body, mesh=mesh, in_specs=in_specs, out_specs=out_specs, check_rep=False
        ),
        donate_argnums=donate,
        keep_unused=True,
    )

    def run(in_maps):
        concat_in = [
            np.concatenate([in_maps[c][nm] for c in range(n_cores)], axis=0)
            for nm in in_names
        ]
        concat_zeros = [
            np.zeros((n_cores * s[0], *s[1:]), d) for (s, d) in zero_shapes
        ]
        out_arrs = sharded(*concat_in, *concat_zeros)
        return [
            {
                nm: np.asarray(out_arrs[i]).reshape(n_cores, *out_avals[i].shape)[c]
                for i, nm in enumerate(out_names)
            }
            for c in range(n_cores)
        ]

    _CACHED[key] = run
    return run


def kernel(pt_src, pt_ref, points_src, points_ref, _repeats=1):
    run = _get_runner(_repeats)
    in_maps = _shard(pt_src, pt_ref, points_src, points_ref)
    results = run(in_maps)
    total = np.float64(0.0)
    for r in results:
        total += np.float64(r["out"].astype(np.float64).sum())
    out = np.float32(total / (4 * 4096))
    return np.asarray(out, dtype=np.float32)


# revision 22
# speedup vs baseline: 201.0663x; 1.1006x over previous
"""Chamfer distance loss kernel for Trainium2 (8 NeuronCores, SPMD).

Problem: nn_ChamferDistLoss — inputs pt_src, pt_ref, points_src, points_ref,
all [B=4, N=4096, 3] fp32.  Output: scalar

    loss = chamfer(pt_src, pt_ref)            # symmetric
         + chamfer_single(pt_src, points_src) # one-sided
         + chamfer_single(pt_ref, points_ref) # one-sided

Every term decomposes into one-sided tasks "sum_i min_j ||a_i - b_j||" over
[4096 x 4096] pairs.  There are 16 (direction, batch) tasks; each core gets 2.

Device algorithm per task (A, B both [4096, 3]):
  d2[i,j] = a2_i + b2_j - 2<a_i,b_j> is computed by ONE K=16 fp16 matmul per
  [128 x 512] tile, using an exact hi/lo fp16 split of the fp32 data (all 4
  cross terms kept), accumulated in fp32 PSUM:
    A rows: [xh yh zh xl yl zl | xh yh zh xl yl zl | 1 1 | a2h a2l]
    B rows: [-2xh..-2zh -2xl..-2zl | -2xl..-2zl -2xh..-2zh | b2h b2l | 1 1]
  Then row-min over chunks (DVE reduce min), relu, sqrt, sum, and a
  partition-sum via ones-matmul -> per-task scalar.  Host: sum 16 / 16384.
"""

import numpy as np

import concourse.bass as bass
import concourse.bacc as bacc
import concourse.tile as tile
from concourse import mybir
from concourse import bass_utils

F32 = mybir.dt.float32
F16 = mybir.dt.float16
AX = mybir.AxisListType
OP = mybir.AluOpType
ACT = mybir.ActivationFunctionType

NPTS = 4096
P = 128
GRP = NPTS // P          # 32 points per partition in the [128, 96] load
NBLK = NPTS // P         # 32 M-blocks per task
CHUNK = 512              # matmul moving free dim
PSUM_FD = 2048           # psum tile free dim (4 banks)
NTILE = NPTS // PSUM_FD  # psum tiles per m-block
NTASK = 2                # tasks per core
K = 16                   # matmul contraction rows


def _row_dma(nc, R, r, src):
    """DMA one [128, 32] SBUF block into row r of R ([1, 4096], n = p*32+c)."""
    nc.gpsimd.dma_start(
        R[r : r + 1, :].rearrange("r (p c) -> r p c", p=P), src
    )


def _build_side(tc, pool, wpool, t_dram, ones_ap, side, tag):
    """Load one [4096, 3] input, build its [16, 4096] fp16 matmul operand."""
    nc = tc.nc
    L = wpool.tile([P, 3 * GRP], F32, tag="ld")
    # contiguous per-partition load: partition p holds points p*32 .. p*32+31
    nc.sync.dma_start(L[:], t_dram.rearrange("(p g) k -> p (g k)", p=P))

    # coordinate source in (k, g) order; b side scaled by -2 (exact in fp16)
    if side == "a":
        base_kg = L[:].rearrange("p (g k) -> p k g", k=3)
    else:
        t2 = wpool.tile([P, 3 * GRP], F32, tag="t2")
        nc.vector.tensor_scalar_mul(t2[:], L[:], -2.0)
        base_kg = t2[:].rearrange("p (g k) -> p k g", k=3)

    # hi/lo fp16 split of the (scaled) coordinates, blocks [xh yh zh xl yl zl]
    S6 = wpool.tile([P, 6 * GRP], F16, tag="s6")
    hi3 = S6[:, 0 : 3 * GRP].rearrange("p (k g) -> p k g", k=3)
    lo3 = S6[:, 3 * GRP : 6 * GRP].rearrange("p (k g) -> p k g", k=3)
    nc.vector.tensor_copy(hi3, base_kg)
    nc.vector.tensor_sub(lo3, base_kg, hi3)

    # squared norm n2 = x^2+y^2+z^2 (fp32), then hi/lo split
    sq = wpool.tile([P, 3 * GRP], F32, tag="sq")
    nc.vector.tensor_mul(sq[:], L[:], L[:])
    sq_kg = sq[:].rearrange("p (g k) -> p k g", k=3)
    n2 = wpool.tile([P, GRP], F32, tag="n2")
    nc.vector.tensor_add(n2[:], sq_kg[:, 0:1, :], sq_kg[:, 1:2, :])
    nc.vector.tensor_add(n2[:], n2[:], sq_kg[:, 2:3, :])
    Sn = wpool.tile([P, 2 * GRP], F16, tag="sn")
    nc.vector.tensor_copy(Sn[:, 0:GRP], n2[:])
    nc.vector.tensor_sub(Sn[:, GRP : 2 * GRP], n2[:], Sn[:, 0:GRP])

    # assemble R [16, 4096] fp16; free index n = p*32 + c
    R = pool.tile([K, NPTS], F16, tag=tag)
    blk = lambda i: S6[:, i * GRP : (i + 1) * GRP]
    if side == "a":
        for r in range(6):
            _row_dma(nc, R, r, blk(r))            # rows 0-5:  h h h l l l
        for r in range(6):
            _row_dma(nc, R, 6 + r, blk(r))        # rows 6-11: h h h l l l
        nc.sync.dma_start(R[12:14, :], ones_ap)   # rows 12-13: 1 1
        _row_dma(nc, R, 14, Sn[:, 0:GRP])         # row 14: a2h
        _row_dma(nc, R, 15, Sn[:, GRP : 2 * GRP])  # row 15: a2l
    else:
        for r in range(6):
            _row_dma(nc, R, r, blk(r))            # rows 0-5:  h h h l l l
        for r in range(3):
            _row_dma(nc, R, 6 + r, blk(3 + r))    # rows 6-8:  l l l
        for r in range(3):
            _row_dma(nc, R, 9 + r, blk(r))        # rows 9-11: h h h
        _row_dma(nc, R, 12, Sn[:, 0:GRP])         # row 12: b2h
        _row_dma(nc, R, 13, Sn[:, GRP : 2 * GRP])  # row 13: b2l
        nc.sync.dma_start(R[14:16, :], ones_ap)   # rows 14-15: 1 1
    return R


def chamfer_core_kernel(tc, out_ap, in_aps, ones_ap):
    """Per-core program: 2 tasks, each sum_i min_j ||a_i - b_j||."""
    nc = tc.nc
    from contextlib import ExitStack

    with ExitStack() as ctx:
        const_pool = ctx.enter_context(tc.tile_pool(name="const", bufs=1))
        work_pool = ctx.enter_context(tc.tile_pool(name="work", bufs=4))
        small_pool = ctx.enter_context(tc.tile_pool(name="small", bufs=4))

        sides = []
        for t in range(NTASK):
            Ra = _build_side(
                tc, const_pool, work_pool, in_aps[2 * t], ones_ap, "a", f"Ra{t}"
            )
            Rb = _build_side(
                tc, const_pool, work_pool, in_aps[2 * t + 1], ones_ap, "b", f"Rb{t}"
            )
            sides.append((Ra, Rb))

        # collect[p, t*32 + m] = min_j d2 for a-point (m*128 + p) of task t
        collect = const_pool.tile([P, NTASK * NBLK], F32)

        # main pool scoped to the loop so the final ones-matmul gets a bank
        with tc.tile_pool(name="psum", bufs=2, space="PSUM") as psum_pool:
            for t in range(NTASK):
                Ra, Rb = sides[t]
                for m in range(NBLK):
                    lhsT = Ra[:, m * P : (m + 1) * P]
                    minv = small_pool.tile([P, NTILE], F32, tag="minv")
                    for i in range(NTILE):
                        ps = psum_pool.tile([P, PSUM_FD], F32, tag="ps")
                        for h in range(PSUM_FD // CHUNK):
                            n0 = i * PSUM_FD + h * CHUNK
                            nc.tensor.matmul(
                                ps[:, h * CHUNK : (h + 1) * CHUNK],
                                lhsT,
                                Rb[:, n0 : n0 + CHUNK],
                                start=True,
                                stop=True,
                            )
                        nc.vector.tensor_reduce(
                            minv[:, i : i + 1], ps[:], axis=AX.X, op=OP.min
                        )
                    nc.vector.tensor_reduce(
                        collect[:, t * NBLK + m : t * NBLK + m + 1],
                        minv[:],
                        axis=AX.X,
                        op=OP.min,
                    )

        # d = sqrt(relu(d2)); per-task per-partition sums
        dists = const_pool.tile([P, NTASK * NBLK], F32)
        nc.vector.tensor_scalar_max(dists[:], collect[:], 0.0)
        nc.scalar.activation(dists[:], dists[:], ACT.Sqrt)

        psums = small_pool.tile([P, NTASK], F32, tag="psums")
        for t in range(NTASK):
            nc.vector.tensor_reduce(
                psums[:, t : t + 1],
                dists[:, t * NBLK : (t + 1) * NBLK],
                axis=AX.X,
                op=OP.add,
            )

        # partition sum via ones-matmul: [1, NTASK] = ones[128,1].T @ psums
        ones = small_pool.tile([P, 1], F32, tag="ones")
        nc.vector.memset(ones[:], 1.0)
        psum_fin_pool = ctx.enter_context(
            tc.tile_pool(name="psum_fin", bufs=1, space="PSUM")
        )
        ps_out = psum_fin_pool.tile([1, CHUNK], F32, tag="ps_fin")
        nc.tensor.matmul(
            ps_out[:, 0:NTASK], ones[:], psums[:], start=True, stop=True
        )
        outs = small_pool.tile([1, NTASK], F32, tag="outs")
        nc.vector.tensor_copy(outs[:], ps_out[:, 0:NTASK])
        nc.gpsimd.dma_start(out_ap, outs[:])


_CACHED = {}


def _get_program(repeats=1):
    if repeats in _CACHED:
        return _CACHED[repeats]
    nc = bacc.Bacc("TRN2", target_bir_lowering=False, debug=False, num_devices=8)
    in_names = ["a0", "b0", "a1", "b1"]
    in_aps = [
        nc.dram_tensor(n, [NPTS, 3], F32, kind="ExternalInput").ap() for n in in_names
    ]
    ones_ap = nc.dram_tensor("ones2", [2, NPTS], F16, kind="ExternalInput").ap()
    out_ap = nc.dram_tensor("out", [1, NTASK], F32, kind="ExternalOutput").ap()
    with tile.TileContext(nc) as tc:
        for _ in range(repeats):
            chamfer_core_kernel(tc, out_ap, in_aps, ones_ap)
    nc.compile()
    _CACHED[repeats] = nc
    return nc


def _shard(pt_src, pt_ref, points_src, points_ref):
    """Host-side sharding: 16 (direction, batch) tasks -> 8 cores x 2 tasks."""
    ones2 = np.ones((2, NPTS), dtype=np.float16)
    in_maps = []
    for c in range(8):
        if c < 4:
            b = c
            m = {"a0": pt_src[b], "b0": pt_ref[b], "a1": pt_ref[b], "b1": pt_src[b]}
        else:
            b = c - 4
            m = {
                "a0": pt_src[b],
                "b0": points_src[b],
                "a1": pt_ref[b],
                "b1": points_ref[b],
            }
        m = {k: np.ascontiguousarray(v, dtype=np.float32) for k, v in m.items()}
        m["ones2"] = ones2
        in_maps.append(m)
    return in_maps


def _get_runner(repeats=1):
    """Cached jitted executor — the NEFF is loaded once; later calls only
    dispatch an execute (unlike run_bass_kernel_spmd, which rebuilds the
    jit closure and re-loads the NEFF on every call)."""
    key = ("runner", repeats)
    if key in _CACHED:
        return _CACHED[key]
    import jax
    from jax.sharding import Mesh, PartitionSpec
    from jax.experimental.shard_map import shard_map
    from concourse import bass2jax, mybir as _mb

    bass2jax.install_neuronx_cc_hook()
    nc = _get_program(repeats)
    n_cores = 8

    partition_name = (
        nc.partition_id_tensor.name if nc.partition_id_tensor is not None else None
    )
    in_names, out_names, out_avals, zero_shapes = [], [], [], []
    for alloc in nc.m.functions[0].allocations:
        if not isinstance(alloc, _mb.MemoryLocationSet):
            continue
        name = alloc.memorylocations[0].name
        if alloc.kind == "ExternalInput":
            if name != partition_name:
                in_names.append(name)
        elif alloc.kind == "ExternalOutput":
            out_names.append(name)
            shape = tuple(alloc.tensor_shape)
            dtype = _mb.dt.np(alloc.dtype)
            out_avals.append(jax.core.ShapedArray(shape, dtype))
            zero_shapes.append((shape, dtype))
    n_params = len(in_names)
    all_names = in_names + out_names
    if partition_name is not None:
        all_names = all_names + [partition_name]
    donate = tuple(range(n_params, n_params + len(out_names)))

    def _body(*args):
        operands = list(args)
        if partition_name is not None:
            operands.append(bass2jax.partition_id_tensor())
        outs = bass2jax._bass_exec_p.bind(
            *operands,
            out_avals=tuple(out_avals),
            in_names=tuple(all_names),
            out_names=tuple(out_names),
            lowering_input_output_aliases=(),
            sim_require_finite=True,
            sim_require_nnan=True,
            nc=nc,
        )
        return tuple(outs)

    devices = jax.devices()[:n_cores]
    mesh = Mesh(np.asarray(devices), ("core",))
    in_specs = (PartitionSpec("core"),) * (n_params + len(out_names))
    out_specs = (PartitionSpec("core"),) * len(out_names)
    sharded = jax.jit(
        shard_map(
            _body, mesh=mesh, in_specs=in_specs, out_specs=out_specs, check_rep=False
        ),
        donate_argnums=donate,
        keep_unused=True,
    )

    def run(in_maps):
        concat_in = [
            np.concatenate([in_maps[c][nm] for c in range(n_cores)], axis=0)
            for nm in in_names
        ]
        concat_zeros = [
            np.zeros((n_cores * s[0], *s[1:]), d) for (s, d) in zero_shapes
        ]
        out_arrs = sharded(*concat_in, *concat_zeros)
        return [
            {
                nm: np.asarray(out_arrs[i]).reshape(n_cores, *out_avals[i].shape)[c]
                for i, nm in enumerate(out_names)
            }
            for c in range(n_cores)
        ]

    _CACHED[key] = run
    return run


def kernel(pt_src, pt_ref, points_src, points_ref, _repeats=1):
    run = _get_runner(_repeats)
    in_maps = _shard(pt_src, pt_ref, points_src, points_ref)
    results = run(in_maps)
    total = np.float64(0.0)
    for r in results:
        total += np.float64(r["out"].astype(np.float64).sum())
    out = np.float32(total / (4 * 4096))
    return np.asarray(out, dtype=np.float32)
